# revision 8
# baseline (speedup 1.0000x reference)
"""Trainium2 Bass kernel for nn_Ensemble (dense MLP ensemble, E=8, B=65536).

v13 = v12 (concurrent ACT+DVE drains via per-engine PSUM tiles) with:

  (i) tick-interleaved phases: instead of [8x L1][8x L2][4x L3] per block
      cycle, emit per tick t: L1 g_t | L2 g_t | (odd t) L3 g_{t//2}.
      The drain engines see a uniform supply -> no phase-edge bubbles,
      and the loop tail (L3-only) shrinks.
 (ii) ACT steals 7 of DVE's 80 b-side drains (ACT 997ns/call vs DVE
      1192ns/call): ACT ~87us ~= DVE ~87us busy, the engine floor.

Background (v12): PSUM is f32; only ACT+DVE can read it (1 elem/cyc/lane).
The Tile scheduler serializes ACT and DVE touching the SAME psum tile, so
each engine gets its own: ps_a (banks 0-3, TOP batch-half -> ACT ->
h1a/h2a/o_a) and ps_b (banks 4-7, BOTTOM half -> DVE -> h1b/h2b/o_b).
PE packs L1's K=64 matmuls in rows (0,0)/(64,0) and L3's M=64 in cols
(0,0)/(0,64) (concurrent in-array), so PE (~56us) is off the critical
path.  Out-DMA ships only the 96 useful rows (6 MiB/core).
"""

import numpy as np
import ml_dtypes

BF16 = ml_dtypes.bfloat16

E = 8
B = 65536
HB = B // 2
IN = 64
AC = 16
H = 128
OUT = 48
OUTP = 64

NT = 512
NB = 4
GW = 1024               # psum cols per engine-group (2 banks)

XW = 2048
XBUFS = 3
OBUFS = 2

# (phase, block, tick) triples whose b-side drain ACT steals from DVE:
# 7 per pass, spread across blocks; phase 1=L1, 2=L2.
_STEAL = {(1, 0, 2), (1, 1, 6), (1, 2, 4), (1, 3, 0),
          (2, 0, 3), (2, 1, 5), (2, 2, 1)}

_CACHED = None


def _build_nc(reps=None):
    import contextlib
    import concourse.bacc as bacc
    import concourse.mybir as mybir
    import concourse.tile as tile

    f32 = mybir.dt.float32
    bf16 = mybir.dt.bfloat16
    AF = mybir.ActivationFunctionType
    ALU = mybir.AluOpType

    nc = bacc.Bacc("TRN2", target_bir_lowering=False)

    x_d = nc.dram_tensor("x", [128, HB], bf16, kind="ExternalInput")
    w1_d = nc.dram_tensor("w1p", [128, H], bf16, kind="ExternalInput")
    w2_d = nc.dram_tensor("w2", [H, H], bf16, kind="ExternalInput")
    w3_d = nc.dram_tensor("w3p", [H, OUTP], bf16, kind="ExternalInput")
    b1_d = nc.dram_tensor("b1v", [H, 1], f32, kind="ExternalInput")
    b2_d = nc.dram_tensor("b2v", [H, 1], f32, kind="ExternalInput")
    b3_d = nc.dram_tensor("b3v", [128, 1], f32, kind="ExternalInput")
    out_d = nc.dram_tensor("out", [96, HB], bf16, kind="ExternalOutput")

    BW = 8192             # x cols per block (HB / NB)
    HW_ = 8192            # h cols per half-block tile

    with tile.TileContext(nc) as tc:
        with (
            tc.tile_pool(name="consts", bufs=1) as consts,
            tc.tile_pool(name="xp", bufs=XBUFS) as xp,
            tc.tile_pool(name="h1a", bufs=2) as h1ap,
            tc.tile_pool(name="h1b", bufs=2) as h1bp,
            tc.tile_pool(name="h2a", bufs=2) as h2ap,
            tc.tile_pool(name="h2b", bufs=2) as h2bp,
            tc.tile_pool(name="oa", bufs=OBUFS) as oap,
            tc.tile_pool(name="ob", bufs=OBUFS) as obp,
            tc.tile_pool(name="psa", bufs=2, space="PSUM") as psap,
            tc.tile_pool(name="psb", bufs=2, space="PSUM") as psbp,
        ):
            w1_sb = consts.tile([128, H], bf16)
            w2_sb = consts.tile([H, H], bf16)
            w3_sb = consts.tile([H, OUTP], bf16)
            b1_sb = consts.tile([H, 1], f32)
            b2_sb = consts.tile([H, 1], f32)
            b3_sb = consts.tile([128, 1], f32)
            nc.sync.dma_start(out=w1_sb, in_=w1_d[:])
            nc.sync.dma_start(out=w2_sb, in_=w2_d[:])
            nc.sync.dma_start(out=w3_sb, in_=w3_d[:])
            nc.sync.dma_start(out=b1_sb, in_=b1_d[:])
            nc.sync.dma_start(out=b2_sb, in_=b2_d[:])
            nc.sync.dma_start(out=b3_sb, in_=b3_d[:])

            def l1_group(x_ts, h1a, h1b, g, blk):
                ps_a = psap.tile([128, GW], f32, name="psa", tag="psa")
                ps_b = psbp.tile([128, GW], f32, name="psb", tag="psb")
                x_t = x_ts[(g * GW) // XW]
                xo = (g * GW) % XW
                for j in range(2):
                    nc.tensor.matmul(
                        ps_a[:, j * NT:(j + 1) * NT], w1_sb[0:64, :],
                        x_t[0:64, xo + j * NT:xo + (j + 1) * NT],
                        start=True, stop=True, tile_position=(0, 0))
                    nc.tensor.matmul(
                        ps_b[:, j * NT:(j + 1) * NT], w1_sb[64:128, :],
                        x_t[64:128, xo + j * NT:xo + (j + 1) * NT],
                        start=True, stop=True, tile_position=(64, 0))
                ho = g * GW
                nc.scalar.activation(h1a[:, ho:ho + GW], ps_a,
                                     AF.Relu, bias=b1_sb)
                if (1, blk, g) in _STEAL:
                    nc.scalar.activation(h1b[:, ho:ho + GW], ps_b,
                                         AF.Relu, bias=b1_sb)
                else:
                    nc.vector.tensor_scalar(h1b[:, ho:ho + GW], ps_b,
                                            b1_sb, 0.0,
                                            op0=ALU.add, op1=ALU.max)

            def l2_group(h1a, h1b, h2a, h2b, g, blk):
                ps_a = psap.tile([128, GW], f32, name="psa", tag="psa")
                ps_b = psbp.tile([128, GW], f32, name="psb", tag="psb")
                ho = g * GW
                for j in range(2):
                    nc.tensor.matmul(
                        ps_a[:, j * NT:(j + 1) * NT], w2_sb,
                        h1a[:, ho + j * NT:ho + (j + 1) * NT],
                        start=True, stop=True)
                    nc.tensor.matmul(
                        ps_b[:, j * NT:(j + 1) * NT], w2_sb,
                        h1b[:, ho + j * NT:ho + (j + 1) * NT],
                        start=True, stop=True)
                nc.scalar.activation(h2a[:, ho:ho + GW], ps_a,
                                     AF.Relu, bias=b2_sb)
                if (2, blk, g) in _STEAL:
                    nc.scalar.activation(h2b[:, ho:ho + GW], ps_b,
                                         AF.Relu, bias=b2_sb)
                else:
                    nc.vector.tensor_scalar(h2b[:, ho:ho + GW], ps_b,
                                            b2_sb, 0.0,
                                            op0=ALU.add, op1=ALU.max)

            def l3_group(h2a, h2b, o_a, o_b, g):
                ps_a = psap.tile([128, GW], f32, name="psa", tag="psa")
                ps_b = psbp.tile([128, GW], f32, name="psb", tag="psb")
                for k, ps in ((0, ps_a), (1, ps_b)):
                    co = (2 * g + k) * GW
                    for j in range(2):
                        nc.tensor.matmul(
                            ps[0:OUTP, j * NT:(j + 1) * NT], w3_sb,
                            h2a[:, co + j * NT:co + (j + 1) * NT],
                            start=True, stop=True, tile_position=(0, 0))
                        nc.tensor.matmul(
                            ps[OUTP:128, j * NT:(j + 1) * NT], w3_sb,
                            h2b[:, co + j * NT:co + (j + 1) * NT],
                            start=True, stop=True, tile_position=(0, OUTP))
                oo = g * GW
                nc.scalar.add(o_a[:, oo:oo + GW], ps_a, b3_sb)
                nc.vector.tensor_scalar_add(o_b[:, oo:oo + GW], ps_b, b3_sb)

            def out_dmas(o_a, o_b, b):
                base = b * BW
                dst = out_d[:, base:base + BW].rearrange(
                    "p (g t) -> p g t", t=2 * GW)
                for rows, orows in ((slice(0, OUT), slice(0, OUT)),
                                    (slice(OUT, 96),
                                     slice(OUTP, OUTP + OUT))):
                    nc.sync.dma_start(
                        out=dst[rows, :, 0:GW],
                        in_=o_a[orows, :].rearrange("p (g t) -> p g t", t=GW))
                    nc.sync.dma_start(
                        out=dst[rows, :, GW:2 * GW],
                        in_=o_b[orows, :].rearrange("p (g t) -> p g t", t=GW))

            loop = (tc.For_i(0, reps, 1, hint_engines=(mybir.EngineType.PE,),
                             staggered_reset=True)
                    if reps is not None else contextlib.nullcontext())
            with loop:
                h1s = {}
                h2s = {}
                for c in range(NB + 2):
                    if c < NB:
                        x_ts = []
                        for k in range(BW // XW):
                            x_t = xp.tile([128, XW], bf16, name="x_t")
                            nc.sync.dma_start(
                                out=x_t,
                                in_=x_d[:, c * BW + k * XW:
                                        c * BW + (k + 1) * XW])
                            x_ts.append(x_t)
                        h1a = h1ap.tile([128, HW_], bf16, name="h1a")
                        h1b = h1bp.tile([128, HW_], bf16, name="h1b")
                        h1s[c] = (h1a, h1b)
                    if 1 <= c <= NB:
                        p1a, p1b = h1s[c - 1]
                        h2a = h2ap.tile([128, HW_], bf16, name="h2a")
                        h2b = h2bp.tile([128, HW_], bf16, name="h2b")
                        h2s[c - 1] = (h2a, h2b)
                    if c >= 2:
                        p2a, p2b = h2s[c - 2]
                        o_a = oap.tile([128, 4096], bf16, name="o_a")
                        o_b = obp.tile([128, 4096], bf16, name="o_b")

                    # interleave the three phases tick by tick
                    for t in range(8):
                        if c < NB:
                            l1_group(x_ts, h1a, h1b, t, c)
                        if 1 <= c <= NB:
                            l2_group(p1a, p1b, h2a, h2b, t, c - 1)
                        if c >= 2 and t % 2 == 1:
                            l3_group(p2a, p2b, o_a, o_b, t // 2)

                    if c >= 2:
                        out_dmas(o_a, o_b, c - 2)
                        h2s.pop(c - 2)
                    if 1 <= c <= NB:
                        h1s.pop(c - 1)

    nc.compile()
    return nc


def _get_nc():
    global _CACHED
    if _CACHED is None:
        _CACHED = _build_nc()
    return _CACHED


def _prep_member(x_e, W1_e, b1_e, W2_e, b2_e, W3_e, b3_e):
    xt = np.ascontiguousarray(np.asarray(x_e).T)      # [64, B] f32
    np.clip(xt[IN - AC:IN], -1.0, 1.0, out=xt[IN - AC:IN])
    X = np.empty((128, HB), dtype=BF16)
    X[0:64] = xt[:, :HB]
    X[64:128] = xt[:, HB:]

    w1p = np.empty((128, H), dtype=BF16)
    w1p[0:64] = W1_e
    w1p[64:128] = W1_e
    w2 = W2_e.astype(BF16)
    w3p = np.zeros((H, OUTP), dtype=BF16)
    w3p[:, :OUT] = W3_e
    b1v = np.ascontiguousarray(b1_e.astype(np.float32).reshape(H, 1))
    b2v = np.ascontiguousarray(b2_e.astype(np.float32).reshape(H, 1))
    b3v = np.zeros((128, 1), dtype=np.float32)
    b3v[0:OUT, 0] = b3_e
    b3v[OUTP:OUTP + OUT, 0] = b3_e
    return {"x": X, "w1p": w1p, "w2": w2, "w3p": w3p,
            "b1v": b1v, "b2v": b2v, "b3v": b3v}


def kernel(**inputs):
    from concourse.bass_utils import run_bass_kernel_spmd

    x = np.asarray(inputs["inputs"], dtype=np.float32).reshape(E, B, IN)
    W1 = np.asarray(inputs["W1"], dtype=np.float32)
    b1 = np.asarray(inputs["b1"], dtype=np.float32)
    W2 = np.asarray(inputs["W2"], dtype=np.float32)
    b2 = np.asarray(inputs["b2"], dtype=np.float32)
    W3 = np.asarray(inputs["W3"], dtype=np.float32)
    b3 = np.asarray(inputs["b3"], dtype=np.float32)

    in_maps = [
        _prep_member(x[e], W1[e], b1[e], W2[e], b2[e], W3[e], b3[e])
        for e in range(E)
    ]

    nc = _get_nc()
    res = run_bass_kernel_spmd(nc, in_maps, core_ids=list(range(E)))

    out = np.empty((E, B, OUT), dtype=np.float32)
    for e in range(E):
        dev = res.results[e]["out"]          # [96, HB] bf16
        out[e, :HB] = dev[0:OUT, :].T
        out[e, HB:] = dev[OUT:96, :].T
    return out


# revision 10
# speedup vs baseline: 1.0006x; 1.0006x over previous
"""Trainium2 Bass kernel for nn_Ensemble (dense MLP ensemble, E=8, B=65536).

v15 = v13 compressed-pipeline over v12 (concurrent ACT+DVE drains via per-engine PSUM tiles) with:

  (i) tick-interleaved phases: instead of [8x L1][8x L2][4x L3] per block
      cycle, emit per tick t: L1 g_t | L2 g_t | (odd t) L3 g_{t//2}.
      The drain engines see a uniform supply -> no phase-edge bubbles,
      and the loop tail (L3-only) shrinks.
 (ii) ACT steals 7 of DVE's 80 b-side drains (ACT 997ns/call vs DVE
      1192ns/call): ACT ~87us ~= DVE ~87us busy, the engine floor.

Background (v12): PSUM is f32; only ACT+DVE can read it (1 elem/cyc/lane).
The Tile scheduler serializes ACT and DVE touching the SAME psum tile, so
each engine gets its own: ps_a (banks 0-3, TOP batch-half -> ACT ->
h1a/h2a/o_a) and ps_b (banks 4-7, BOTTOM half -> DVE -> h1b/h2b/o_b).
PE packs L1's K=64 matmuls in rows (0,0)/(64,0) and L3's M=64 in cols
(0,0)/(0,64) (concurrent in-array), so PE (~56us) is off the critical
path.  Out-DMA ships only the 96 useful rows (6 MiB/core).
"""

import numpy as np
import ml_dtypes

BF16 = ml_dtypes.bfloat16

E = 8
B = 65536
HB = B // 2
IN = 64
AC = 16
H = 128
OUT = 48
OUTP = 64

NT = 512
NB = 4
GW = 1024               # psum cols per engine-group (2 banks)

XW = 4096
XBUFS = 3
OBUFS = 2

# (phase, block, tick) triples whose b-side drain ACT steals from DVE:
# 7 per pass, spread across blocks; phase 1=L1, 2=L2.
_STEAL = {(1, 0, 2), (1, 1, 6), (1, 2, 4), (1, 3, 0),
          (2, 0, 3), (2, 1, 5), (2, 2, 1)}

_CACHED = None


def _build_nc(reps=None):
    import contextlib
    import concourse.bacc as bacc
    import concourse.mybir as mybir
    import concourse.tile as tile

    f32 = mybir.dt.float32
    bf16 = mybir.dt.bfloat16
    AF = mybir.ActivationFunctionType
    ALU = mybir.AluOpType

    nc = bacc.Bacc("TRN2", target_bir_lowering=False)

    x_d = nc.dram_tensor("x", [128, HB], bf16, kind="ExternalInput")
    w1_d = nc.dram_tensor("w1p", [128, H], bf16, kind="ExternalInput")
    w2_d = nc.dram_tensor("w2", [H, H], bf16, kind="ExternalInput")
    w3_d = nc.dram_tensor("w3p", [H, OUTP], bf16, kind="ExternalInput")
    b1_d = nc.dram_tensor("b1v", [H, 1], f32, kind="ExternalInput")
    b2_d = nc.dram_tensor("b2v", [H, 1], f32, kind="ExternalInput")
    b3_d = nc.dram_tensor("b3v", [128, 1], f32, kind="ExternalInput")
    out_d = nc.dram_tensor("out", [96, HB], bf16, kind="ExternalOutput")

    BW = 8192             # x cols per block (HB / NB)
    HW_ = 8192            # h cols per half-block tile

    with tile.TileContext(nc) as tc:
        with (
            tc.tile_pool(name="consts", bufs=1) as consts,
            tc.tile_pool(name="xp", bufs=XBUFS) as xp,
            tc.tile_pool(name="h1a", bufs=2) as h1ap,
            tc.tile_pool(name="h1b", bufs=2) as h1bp,
            tc.tile_pool(name="h2a", bufs=2) as h2ap,
            tc.tile_pool(name="h2b", bufs=2) as h2bp,
            tc.tile_pool(name="oa", bufs=OBUFS) as oap,
            tc.tile_pool(name="ob", bufs=OBUFS) as obp,
            tc.tile_pool(name="psa", bufs=2, space="PSUM") as psap,
            tc.tile_pool(name="psb", bufs=2, space="PSUM") as psbp,
        ):
            w1_sb = consts.tile([128, H], bf16)
            w2_sb = consts.tile([H, H], bf16)
            w3_sb = consts.tile([H, OUTP], bf16)
            b1_sb = consts.tile([H, 1], f32)
            b2_sb = consts.tile([H, 1], f32)
            b3_sb = consts.tile([128, 1], f32)
            nc.sync.dma_start(out=w1_sb, in_=w1_d[:])
            nc.sync.dma_start(out=w2_sb, in_=w2_d[:])
            nc.sync.dma_start(out=w3_sb, in_=w3_d[:])
            nc.sync.dma_start(out=b1_sb, in_=b1_d[:])
            nc.sync.dma_start(out=b2_sb, in_=b2_d[:])
            nc.sync.dma_start(out=b3_sb, in_=b3_d[:])

            def l1_group(x_ts, h1a, h1b, g, blk):
                ps_a = psap.tile([128, GW], f32, name="psa", tag="psa")
                ps_b = psbp.tile([128, GW], f32, name="psb", tag="psb")
                x_t = x_ts[(g * GW) // XW]
                xo = (g * GW) % XW
                for j in range(2):
                    nc.tensor.matmul(
                        ps_a[:, j * NT:(j + 1) * NT], w1_sb[0:64, :],
                        x_t[0:64, xo + j * NT:xo + (j + 1) * NT],
                        start=True, stop=True, tile_position=(0, 0))
                    nc.tensor.matmul(
                        ps_b[:, j * NT:(j + 1) * NT], w1_sb[64:128, :],
                        x_t[64:128, xo + j * NT:xo + (j + 1) * NT],
                        start=True, stop=True, tile_position=(64, 0))
                ho = g * GW
                nc.scalar.activation(h1a[:, ho:ho + GW], ps_a,
                                     AF.Relu, bias=b1_sb)
                if (1, blk, g) in _STEAL:
                    nc.scalar.activation(h1b[:, ho:ho + GW], ps_b,
                                         AF.Relu, bias=b1_sb)
                else:
                    nc.vector.tensor_scalar(h1b[:, ho:ho + GW], ps_b,
                                            b1_sb, 0.0,
                                            op0=ALU.add, op1=ALU.max)

            def l2_group(h1a, h1b, h2a, h2b, g, blk):
                ps_a = psap.tile([128, GW], f32, name="psa", tag="psa")
                ps_b = psbp.tile([128, GW], f32, name="psb", tag="psb")
                ho = g * GW
                for j in range(2):
                    nc.tensor.matmul(
                        ps_a[:, j * NT:(j + 1) * NT], w2_sb,
                        h1a[:, ho + j * NT:ho + (j + 1) * NT],
                        start=True, stop=True)
                    nc.tensor.matmul(
                        ps_b[:, j * NT:(j + 1) * NT], w2_sb,
                        h1b[:, ho + j * NT:ho + (j + 1) * NT],
                        start=True, stop=True)
                nc.scalar.activation(h2a[:, ho:ho + GW], ps_a,
                                     AF.Relu, bias=b2_sb)
                if (2, blk, g) in _STEAL:
                    nc.scalar.activation(h2b[:, ho:ho + GW], ps_b,
                                         AF.Relu, bias=b2_sb)
                else:
                    nc.vector.tensor_scalar(h2b[:, ho:ho + GW], ps_b,
                                            b2_sb, 0.0,
                                            op0=ALU.add, op1=ALU.max)

            def l3_half(h2a, h2b, o_a, o_b, q):
                g, k = divmod(q, 2)
                if k == 0:
                    ps = psap.tile([128, GW], f32, name="psa", tag="psa")
                else:
                    ps = psbp.tile([128, GW], f32, name="psb", tag="psb")
                co = q * GW
                for j in range(2):
                    nc.tensor.matmul(
                        ps[0:OUTP, j * NT:(j + 1) * NT], w3_sb,
                        h2a[:, co + j * NT:co + (j + 1) * NT],
                        start=True, stop=True, tile_position=(0, 0))
                    nc.tensor.matmul(
                        ps[OUTP:128, j * NT:(j + 1) * NT], w3_sb,
                        h2b[:, co + j * NT:co + (j + 1) * NT],
                        start=True, stop=True, tile_position=(0, OUTP))
                oo = g * GW
                if k == 0:
                    nc.scalar.add(o_a[:, oo:oo + GW], ps, b3_sb)
                else:
                    nc.vector.tensor_scalar_add(o_b[:, oo:oo + GW], ps, b3_sb)

            def out_dmas(o_a, o_b, b):
                base = b * BW
                dst = out_d[:, base:base + BW].rearrange(
                    "p (g t) -> p g t", t=2 * GW)
                for rows, orows in ((slice(0, OUT), slice(0, OUT)),
                                    (slice(OUT, 96),
                                     slice(OUTP, OUTP + OUT))):
                    nc.sync.dma_start(
                        out=dst[rows, :, 0:GW],
                        in_=o_a[orows, :].rearrange("p (g t) -> p g t", t=GW))
                    nc.sync.dma_start(
                        out=dst[rows, :, GW:2 * GW],
                        in_=o_b[orows, :].rearrange("p (g t) -> p g t", t=GW))

            loop = (tc.For_i(0, reps, 1, hint_engines=(mybir.EngineType.PE,))
                    if reps is not None else contextlib.nullcontext())
            with loop:
                # Compressed pipeline: one global tick stream; L2 trails L1
                # by 2 ticks, L3 halves trail their L2 producers by 2.
                # Global tick u = c * 8 + t; L1 tick u covers (block u//8,
                # group u%8); L2 at u covers L1's tick u-2; L3 half at u
                # covers h2 chunk of tick u-4 (a-half even, b-half odd).
                hs = {}   # block -> (h1a, h1b, h2a, h2b)
                os_ = {}  # block -> (o_a, o_b)
                NU = NB * 8
                for u in range(NU + 6):
                    c, t = divmod(u, 8)
                    if t == 0 and c < NB:
                        x_ts = []
                        for k in range(BW // XW):
                            x_t = xp.tile([128, XW], bf16, name="x_t")
                            nc.sync.dma_start(
                                out=x_t,
                                in_=x_d[:, c * BW + k * XW:
                                        c * BW + (k + 1) * XW])
                            x_ts.append(x_t)
                        hs[c] = (h1ap.tile([128, HW_], bf16, name="h1a"),
                                 h1bp.tile([128, HW_], bf16, name="h1b"),
                                 h2ap.tile([128, HW_], bf16, name="h2a"),
                                 h2bp.tile([128, HW_], bf16, name="h2b"),
                                 x_ts)
                        os_[c] = (oap.tile([128, 4096], bf16, name="o_a"),
                                  obp.tile([128, 4096], bf16, name="o_b"))
                    if u < NU:
                        h1a, h1b, _, _, x_ts = hs[c]
                        l1_group(x_ts, h1a, h1b, t, c)
                    u2 = u - 2
                    if 0 <= u2 < NU:
                        c2, t2 = divmod(u2, 8)
                        h1a, h1b, h2a, h2b, _ = hs[c2]
                        l2_group(h1a, h1b, h2a, h2b, t2, c2)
                    u3 = u - 4
                    if 0 <= u3 < NU:
                        c3, q = divmod(u3, 8)
                        _, _, h2a, h2b, _ = hs[c3]
                        o_a, o_b = os_[c3]
                        l3_half(h2a, h2b, o_a, o_b, q)
                        if q == 7:
                            out_dmas(o_a, o_b, c3)
                            os_.pop(c3)
                            hs.pop(c3)

    nc.compile()
    return nc


def _get_nc():
    global _CACHED
    if _CACHED is None:
        _CACHED = _build_nc()
    return _CACHED


def _prep_member(x_e, W1_e, b1_e, W2_e, b2_e, W3_e, b3_e):
    xt = np.ascontiguousarray(np.asarray(x_e).T)      # [64, B] f32
    np.clip(xt[IN - AC:IN], -1.0, 1.0, out=xt[IN - AC:IN])
    X = np.empty((128, HB), dtype=BF16)
    X[0:64] = xt[:, :HB]
    X[64:128] = xt[:, HB:]

    w1p = np.empty((128, H), dtype=BF16)
    w1p[0:64] = W1_e
    w1p[64:128] = W1_e
    w2 = W2_e.astype(BF16)
    w3p = np.zeros((H, OUTP), dtype=BF16)
    w3p[:, :OUT] = W3_e
    b1v = np.ascontiguousarray(b1_e.astype(np.float32).reshape(H, 1))
    b2v = np.ascontiguousarray(b2_e.astype(np.float32).reshape(H, 1))
    b3v = np.zeros((128, 1), dtype=np.float32)
    b3v[0:OUT, 0] = b3_e
    b3v[OUTP:OUTP + OUT, 0] = b3_e
    return {"x": X, "w1p": w1p, "w2": w2, "w3p": w3p,
            "b1v": b1v, "b2v": b2v, "b3v": b3v}


def kernel(**inputs):
    from concourse.bass_utils import run_bass_kernel_spmd

    x = np.asarray(inputs["inputs"], dtype=np.float32).reshape(E, B, IN)
    W1 = np.asarray(inputs["W1"], dtype=np.float32)
    b1 = np.asarray(inputs["b1"], dtype=np.float32)
    W2 = np.asarray(inputs["W2"], dtype=np.float32)
    b2 = np.asarray(inputs["b2"], dtype=np.float32)
    W3 = np.asarray(inputs["W3"], dtype=np.float32)
    b3 = np.asarray(inputs["b3"], dtype=np.float32)

    in_maps = [
        _prep_member(x[e], W1[e], b1[e], W2[e], b2[e], W3[e], b3[e])
        for e in range(E)
    ]

    nc = _get_nc()
    res = run_bass_kernel_spmd(nc, in_maps, core_ids=list(range(E)))

    out = np.empty((E, B, OUT), dtype=np.float32)
    for e in range(E):
        dev = res.results[e]["out"]          # [96, HB] bf16
        out[e, :HB] = dev[0:OUT, :].T
        out[e, HB:] = dev[OUT:96, :].T
    return out


# revision 11
# speedup vs baseline: 1.3349x; 1.3341x over previous
"""Trainium2 Bass kernel for nn_Ensemble (dense MLP ensemble, E=8, B=65536).

v15: compressed software pipeline + concurrent ACT/DVE drains.

Bottleneck analysis: PSUM is f32 and only ACT (1.2GHz) and DVE (0.96GHz)
can read it, 1 elem/cycle/lane (DMA and GPSIMD have no PSUM port), so the
PSUM->SBUF drains (relu+cast, 163840 cols/member) set a ~87us floor.  The
Tile scheduler serializes ACT and DVE touching the SAME psum tile, so each
engine owns its own psum tiles and SBUF destinations:

  - ps_a (banks 0-3): TOP batch-half, ACT -> h1a/h2a/o_a
  - ps_b (banks 4-7): BOTTOM batch-half, DVE -> h1b/h2b/o_b

Structure: one global tick stream u = 0..37; tick u runs L1 group (u//8,
u%8), the L2 group trailing it by 2 ticks, and the L3 half-group trailing
by 4 -- all three layers advance concurrently, the drain engines see a
uniform 2.5-call/tick supply, and pipeline ramp/tail is ~4 ticks instead
of 2 full block-phases.  ACT "steals" 7 of DVE's 80 drains to balance
busy time (~87us each).  PE stays off the critical path (~56us): L1's
K=64 matmul pairs pack rows (0,0)/(64,0), L3's M=64 pairs pack cols
(0,0)/(0,64) and run concurrently in the array; only L2 (K=128)
serializes.  Out-DMA ships only the 96 useful rows (6 MiB/core), with
o_a/o_b 1024-col chunks interleaved in HBM via 3D (rearranged) DMA APs.

Sharding: one ensemble member per NeuronCore (E=8 = n_cores); weights are
tiny and per-member, so cores are fully independent (no collectives).
Measured: 160us (v10 baseline) -> ~110us, rel_err 4.2e-3.
"""

import numpy as np
import ml_dtypes

BF16 = ml_dtypes.bfloat16

E = 8
B = 65536
HB = B // 2
IN = 64
AC = 16
H = 128
OUT = 48
OUTP = 64

NT = 512
NB = 4
GW = 1024               # psum cols per engine-group (2 banks)

XW = 4096
XBUFS = 3
OBUFS = 2

# (phase, block, tick) triples whose b-side drain ACT steals from DVE:
# 7 per pass, spread across blocks; phase 1=L1, 2=L2.
_STEAL = {(1, 0, 2), (1, 1, 6), (1, 2, 4), (1, 3, 0),
          (2, 0, 3), (2, 1, 5), (2, 2, 1)}

_CACHED = None


def _build_nc(reps=None):
    import contextlib
    import concourse.bacc as bacc
    import concourse.mybir as mybir
    import concourse.tile as tile

    f32 = mybir.dt.float32
    bf16 = mybir.dt.bfloat16
    AF = mybir.ActivationFunctionType
    ALU = mybir.AluOpType

    nc = bacc.Bacc("TRN2", target_bir_lowering=False)

    x_d = nc.dram_tensor("x", [128, HB], bf16, kind="ExternalInput")
    w1_d = nc.dram_tensor("w1p", [128, H], bf16, kind="ExternalInput")
    w2_d = nc.dram_tensor("w2", [H, H], bf16, kind="ExternalInput")
    w3_d = nc.dram_tensor("w3p", [H, OUTP], bf16, kind="ExternalInput")
    b1_d = nc.dram_tensor("b1v", [H, 1], f32, kind="ExternalInput")
    b2_d = nc.dram_tensor("b2v", [H, 1], f32, kind="ExternalInput")
    b3_d = nc.dram_tensor("b3v", [128, 1], f32, kind="ExternalInput")
    out_d = nc.dram_tensor("out", [96, HB], bf16, kind="ExternalOutput")

    BW = 8192             # x cols per block (HB / NB)
    HW_ = 8192            # h cols per half-block tile

    with tile.TileContext(nc) as tc:
        with (
            tc.tile_pool(name="consts", bufs=1) as consts,
            tc.tile_pool(name="xp", bufs=XBUFS) as xp,
            tc.tile_pool(name="h1a", bufs=2) as h1ap,
            tc.tile_pool(name="h1b", bufs=2) as h1bp,
            tc.tile_pool(name="h2a", bufs=2) as h2ap,
            tc.tile_pool(name="h2b", bufs=2) as h2bp,
            tc.tile_pool(name="oa", bufs=OBUFS) as oap,
            tc.tile_pool(name="ob", bufs=OBUFS) as obp,
            tc.tile_pool(name="psa", bufs=2, space="PSUM") as psap,
            tc.tile_pool(name="psb", bufs=2, space="PSUM") as psbp,
        ):
            w1_sb = consts.tile([128, H], bf16)
            w2_sb = consts.tile([H, H], bf16)
            w3_sb = consts.tile([H, OUTP], bf16)
            b1_sb = consts.tile([H, 1], f32)
            b2_sb = consts.tile([H, 1], f32)
            b3_sb = consts.tile([128, 1], f32)
            nc.sync.dma_start(out=w1_sb, in_=w1_d[:])
            nc.sync.dma_start(out=w2_sb, in_=w2_d[:])
            nc.sync.dma_start(out=w3_sb, in_=w3_d[:])
            nc.sync.dma_start(out=b1_sb, in_=b1_d[:])
            nc.sync.dma_start(out=b2_sb, in_=b2_d[:])
            nc.sync.dma_start(out=b3_sb, in_=b3_d[:])

            def l1_group(x_ts, h1a, h1b, g, blk):
                ps_a = psap.tile([128, GW], f32, name="psa", tag="psa")
                ps_b = psbp.tile([128, GW], f32, name="psb", tag="psb")
                x_t = x_ts[(g * GW) // XW]
                xo = (g * GW) % XW
                for j in range(2):
                    nc.tensor.matmul(
                        ps_a[:, j * NT:(j + 1) * NT], w1_sb[0:64, :],
                        x_t[0:64, xo + j * NT:xo + (j + 1) * NT],
                        start=True, stop=True, tile_position=(0, 0))
                    nc.tensor.matmul(
                        ps_b[:, j * NT:(j + 1) * NT], w1_sb[64:128, :],
                        x_t[64:128, xo + j * NT:xo + (j + 1) * NT],
                        start=True, stop=True, tile_position=(64, 0))
                ho = g * GW
                nc.scalar.activation(h1a[:, ho:ho + GW], ps_a,
                                     AF.Relu, bias=b1_sb)
                if (1, blk, g) in _STEAL:
                    nc.scalar.activation(h1b[:, ho:ho + GW], ps_b,
                                         AF.Relu, bias=b1_sb)
                else:
                    nc.vector.tensor_scalar(h1b[:, ho:ho + GW], ps_b,
                                            b1_sb, 0.0,
                                            op0=ALU.add, op1=ALU.max)

            def l2_group(h1a, h1b, h2a, h2b, g, blk):
                ps_a = psap.tile([128, GW], f32, name="psa", tag="psa")
                ps_b = psbp.tile([128, GW], f32, name="psb", tag="psb")
                ho = g * GW
                for j in range(2):
                    nc.tensor.matmul(
                        ps_a[:, j * NT:(j + 1) * NT], w2_sb,
                        h1a[:, ho + j * NT:ho + (j + 1) * NT],
                        start=True, stop=True)
                    nc.tensor.matmul(
                        ps_b[:, j * NT:(j + 1) * NT], w2_sb,
                        h1b[:, ho + j * NT:ho + (j + 1) * NT],
                        start=True, stop=True)
                nc.scalar.activation(h2a[:, ho:ho + GW], ps_a,
                                     AF.Relu, bias=b2_sb)
                if (2, blk, g) in _STEAL:
                    nc.scalar.activation(h2b[:, ho:ho + GW], ps_b,
                                         AF.Relu, bias=b2_sb)
                else:
                    nc.vector.tensor_scalar(h2b[:, ho:ho + GW], ps_b,
                                            b2_sb, 0.0,
                                            op0=ALU.add, op1=ALU.max)

            def l3_half(h2a, h2b, o_a, o_b, q):
                g, k = divmod(q, 2)
                if k == 0:
                    ps = psap.tile([128, GW], f32, name="psa", tag="psa")
                else:
                    ps = psbp.tile([128, GW], f32, name="psb", tag="psb")
                co = q * GW
                for j in range(2):
                    nc.tensor.matmul(
                        ps[0:OUTP, j * NT:(j + 1) * NT], w3_sb,
                        h2a[:, co + j * NT:co + (j + 1) * NT],
                        start=True, stop=True, tile_position=(0, 0))
                    nc.tensor.matmul(
                        ps[OUTP:128, j * NT:(j + 1) * NT], w3_sb,
                        h2b[:, co + j * NT:co + (j + 1) * NT],
                        start=True, stop=True, tile_position=(0, OUTP))
                oo = g * GW
                if k == 0:
                    nc.scalar.add(o_a[:, oo:oo + GW], ps, b3_sb)
                else:
                    nc.vector.tensor_scalar_add(o_b[:, oo:oo + GW], ps, b3_sb)

            def out_dmas(o_a, o_b, b):
                base = b * BW
                dst = out_d[:, base:base + BW].rearrange(
                    "p (g t) -> p g t", t=2 * GW)
                for rows, orows in ((slice(0, OUT), slice(0, OUT)),
                                    (slice(OUT, 96),
                                     slice(OUTP, OUTP + OUT))):
                    nc.sync.dma_start(
                        out=dst[rows, :, 0:GW],
                        in_=o_a[orows, :].rearrange("p (g t) -> p g t", t=GW))
                    nc.sync.dma_start(
                        out=dst[rows, :, GW:2 * GW],
                        in_=o_b[orows, :].rearrange("p (g t) -> p g t", t=GW))

            loop = (tc.For_i(0, reps, 1, hint_engines=(mybir.EngineType.PE,))
                    if reps is not None else contextlib.nullcontext())
            with loop:
                # Compressed pipeline: one global tick stream; L2 trails L1
                # by 2 ticks, L3 halves trail their L2 producers by 2.
                # Global tick u = c * 8 + t; L1 tick u covers (block u//8,
                # group u%8); L2 at u covers L1's tick u-2; L3 half at u
                # covers h2 chunk of tick u-4 (a-half even, b-half odd).
                hs = {}   # block -> (h1a, h1b, h2a, h2b)
                os_ = {}  # block -> (o_a, o_b)
                NU = NB * 8
                for u in range(NU + 6):
                    c, t = divmod(u, 8)
                    if t == 0 and c < NB:
                        x_ts = []
                        for k in range(BW // XW):
                            x_t = xp.tile([128, XW], bf16, name="x_t")
                            nc.sync.dma_start(
                                out=x_t,
                                in_=x_d[:, c * BW + k * XW:
                                        c * BW + (k + 1) * XW])
                            x_ts.append(x_t)
                        hs[c] = (h1ap.tile([128, HW_], bf16, name="h1a"),
                                 h1bp.tile([128, HW_], bf16, name="h1b"),
                                 h2ap.tile([128, HW_], bf16, name="h2a"),
                                 h2bp.tile([128, HW_], bf16, name="h2b"),
                                 x_ts)
                        os_[c] = (oap.tile([128, 4096], bf16, name="o_a"),
                                  obp.tile([128, 4096], bf16, name="o_b"))
                    if u < NU:
                        h1a, h1b, _, _, x_ts = hs[c]
                        l1_group(x_ts, h1a, h1b, t, c)
                    u2 = u - 2
                    if 0 <= u2 < NU:
                        c2, t2 = divmod(u2, 8)
                        h1a, h1b, h2a, h2b, _ = hs[c2]
                        l2_group(h1a, h1b, h2a, h2b, t2, c2)
                    u3 = u - 4
                    if 0 <= u3 < NU:
                        c3, q = divmod(u3, 8)
                        _, _, h2a, h2b, _ = hs[c3]
                        o_a, o_b = os_[c3]
                        l3_half(h2a, h2b, o_a, o_b, q)
                        if q == 7:
                            out_dmas(o_a, o_b, c3)
                            os_.pop(c3)
                            hs.pop(c3)

    nc.compile()
    return nc


def _get_nc():
    global _CACHED
    if _CACHED is None:
        _CACHED = _build_nc()
    return _CACHED


def _prep_member(x_e, W1_e, b1_e, W2_e, b2_e, W3_e, b3_e):
    xt = np.ascontiguousarray(np.asarray(x_e).T)      # [64, B] f32
    np.clip(xt[IN - AC:IN], -1.0, 1.0, out=xt[IN - AC:IN])
    X = np.empty((128, HB), dtype=BF16)
    X[0:64] = xt[:, :HB]
    X[64:128] = xt[:, HB:]

    w1p = np.empty((128, H), dtype=BF16)
    w1p[0:64] = W1_e
    w1p[64:128] = W1_e
    w2 = W2_e.astype(BF16)
    w3p = np.zeros((H, OUTP), dtype=BF16)
    w3p[:, :OUT] = W3_e
    b1v = np.ascontiguousarray(b1_e.astype(np.float32).reshape(H, 1))
    b2v = np.ascontiguousarray(b2_e.astype(np.float32).reshape(H, 1))
    b3v = np.zeros((128, 1), dtype=np.float32)
    b3v[0:OUT, 0] = b3_e
    b3v[OUTP:OUTP + OUT, 0] = b3_e
    return {"x": X, "w1p": w1p, "w2": w2, "w3p": w3p,
            "b1v": b1v, "b2v": b2v, "b3v": b3v}


def kernel(**inputs):
    from concourse.bass_utils import run_bass_kernel_spmd

    x = np.asarray(inputs["inputs"], dtype=np.float32).reshape(E, B, IN)
    W1 = np.asarray(inputs["W1"], dtype=np.float32)
    b1 = np.asarray(inputs["b1"], dtype=np.float32)
    W2 = np.asarray(inputs["W2"], dtype=np.float32)
    b2 = np.asarray(inputs["b2"], dtype=np.float32)
    W3 = np.asarray(inputs["W3"], dtype=np.float32)
    b3 = np.asarray(inputs["b3"], dtype=np.float32)

    in_maps = [
        _prep_member(x[e], W1[e], b1[e], W2[e], b2[e], W3[e], b3[e])
        for e in range(E)
    ]

    nc = _get_nc()
    res = run_bass_kernel_spmd(nc, in_maps, core_ids=list(range(E)))

    out = np.empty((E, B, OUT), dtype=np.float32)
    for e in range(E):
        dev = res.results[e]["out"]          # [96, HB] bf16
        out[e, :HB] = dev[0:OUT, :].T
        out[e, HB:] = dev[OUT:96, :].T
    return out


# revision 12
# speedup vs baseline: 1.6971x; 1.2713x over previous
"""Trainium2 Bass kernel for nn_Ensemble (dense MLP ensemble, E=8, B=65536).

v15: compressed software pipeline + concurrent ACT/DVE drains.

Bottleneck analysis: PSUM is f32 and only ACT (1.2GHz) and DVE (0.96GHz)
can read it, 1 elem/cycle/lane (DMA and GPSIMD have no PSUM port), so the
PSUM->SBUF drains (relu+cast, 163840 cols/member) set a ~87us floor.  The
Tile scheduler serializes ACT and DVE touching the SAME psum tile, so each
engine owns its own psum tiles and SBUF destinations:

  - ps_a (banks 0-3): TOP batch-half, ACT -> h1a/h2a/o_a
  - ps_b (banks 4-7): BOTTOM batch-half, DVE -> h1b/h2b/o_b

Structure: one global tick stream u = 0..37; tick u runs L1 group (u//8,
u%8), the L2 group trailing it by 2 ticks, and the L3 half-group trailing
by 4 -- all three layers advance concurrently, the drain engines see a
uniform 2.5-call/tick supply, and pipeline ramp/tail is ~4 ticks instead
of 2 full block-phases.  ACT "steals" 7 of DVE's 80 drains to balance
busy time (~87us each).  PE stays off the critical path (~56us): L1's
K=64 matmul pairs pack rows (0,0)/(64,0), L3's M=64 pairs pack cols
(0,0)/(0,64) and run concurrently in the array; only L2 (K=128)
serializes.  Out-DMA ships only the 96 useful rows (6 MiB/core), with
o_a/o_b 1024-col chunks interleaved in HBM via 3D (rearranged) DMA APs;
x in-DMAs are split into row-halves (parallel queues) and the final
block's out-chunks ship as they drain so the loop tail is one DMA deep.

Sharding: one ensemble member per NeuronCore (E=8 = n_cores); weights are
tiny and per-member, so cores are fully independent (no collectives).
Measured: 160us (v10 baseline) -> ~110us, rel_err 4.2e-3.
"""

import numpy as np
import ml_dtypes

BF16 = ml_dtypes.bfloat16

E = 8
B = 65536
HB = B // 2
IN = 64
AC = 16
H = 128
OUT = 48
OUTP = 64

NT = 512
NB = 4
GW = 1024               # psum cols per engine-group (2 banks)

XW = 4096
XBUFS = 3
OBUFS = 2

# (phase, block, tick) triples whose b-side drain ACT steals from DVE:
# 7 per pass, spread across blocks; phase 1=L1, 2=L2.
_STEAL = {(1, 0, 2), (1, 1, 6), (1, 2, 4), (1, 3, 0),
          (2, 0, 3), (2, 1, 5), (2, 2, 1)}

_CACHED = None


def _build_nc(reps=None):
    import contextlib
    import concourse.bacc as bacc
    import concourse.mybir as mybir
    import concourse.tile as tile

    f32 = mybir.dt.float32
    bf16 = mybir.dt.bfloat16
    AF = mybir.ActivationFunctionType
    ALU = mybir.AluOpType

    nc = bacc.Bacc("TRN2", target_bir_lowering=False)

    x_d = nc.dram_tensor("x", [128, HB], bf16, kind="ExternalInput")
    w1_d = nc.dram_tensor("w1p", [128, H], bf16, kind="ExternalInput")
    w2_d = nc.dram_tensor("w2", [H, H], bf16, kind="ExternalInput")
    w3_d = nc.dram_tensor("w3p", [H, OUTP], bf16, kind="ExternalInput")
    b1_d = nc.dram_tensor("b1v", [H, 1], f32, kind="ExternalInput")
    b2_d = nc.dram_tensor("b2v", [H, 1], f32, kind="ExternalInput")
    b3_d = nc.dram_tensor("b3v", [128, 1], f32, kind="ExternalInput")
    out_d = nc.dram_tensor("out", [96, HB], bf16, kind="ExternalOutput")

    BW = 8192             # x cols per block (HB / NB)
    HW_ = 8192            # h cols per half-block tile

    with tile.TileContext(nc) as tc:
        with (
            tc.tile_pool(name="consts", bufs=1) as consts,
            tc.tile_pool(name="xp", bufs=XBUFS) as xp,
            tc.tile_pool(name="h1a", bufs=2) as h1ap,
            tc.tile_pool(name="h1b", bufs=2) as h1bp,
            tc.tile_pool(name="h2a", bufs=2) as h2ap,
            tc.tile_pool(name="h2b", bufs=2) as h2bp,
            tc.tile_pool(name="oa", bufs=OBUFS) as oap,
            tc.tile_pool(name="ob", bufs=OBUFS) as obp,
            tc.tile_pool(name="psa", bufs=2, space="PSUM") as psap,
            tc.tile_pool(name="psb", bufs=2, space="PSUM") as psbp,
        ):
            w1_sb = consts.tile([128, H], bf16)
            w2_sb = consts.tile([H, H], bf16)
            w3_sb = consts.tile([H, OUTP], bf16)
            b1_sb = consts.tile([H, 1], f32)
            b2_sb = consts.tile([H, 1], f32)
            b3_sb = consts.tile([128, 1], f32)
            nc.sync.dma_start(out=w1_sb, in_=w1_d[:])
            nc.sync.dma_start(out=w2_sb, in_=w2_d[:])
            nc.sync.dma_start(out=w3_sb, in_=w3_d[:])
            nc.sync.dma_start(out=b1_sb, in_=b1_d[:])
            nc.sync.dma_start(out=b2_sb, in_=b2_d[:])
            nc.sync.dma_start(out=b3_sb, in_=b3_d[:])

            def l1_group(x_ts, h1a, h1b, g, blk):
                ps_a = psap.tile([128, GW], f32, name="psa", tag="psa")
                ps_b = psbp.tile([128, GW], f32, name="psb", tag="psb")
                x_t = x_ts[(g * GW) // XW]
                xo = (g * GW) % XW
                for j in range(2):
                    nc.tensor.matmul(
                        ps_a[:, j * NT:(j + 1) * NT], w1_sb[0:64, :],
                        x_t[0:64, xo + j * NT:xo + (j + 1) * NT],
                        start=True, stop=True, tile_position=(0, 0))
                    nc.tensor.matmul(
                        ps_b[:, j * NT:(j + 1) * NT], w1_sb[64:128, :],
                        x_t[64:128, xo + j * NT:xo + (j + 1) * NT],
                        start=True, stop=True, tile_position=(64, 0))
                ho = g * GW
                nc.scalar.activation(h1a[:, ho:ho + GW], ps_a,
                                     AF.Relu, bias=b1_sb)
                if (1, blk, g) in _STEAL:
                    nc.scalar.activation(h1b[:, ho:ho + GW], ps_b,
                                         AF.Relu, bias=b1_sb)
                else:
                    nc.vector.tensor_scalar(h1b[:, ho:ho + GW], ps_b,
                                            b1_sb, 0.0,
                                            op0=ALU.add, op1=ALU.max)

            def l2_group(h1a, h1b, h2a, h2b, g, blk):
                ps_a = psap.tile([128, GW], f32, name="psa", tag="psa")
                ps_b = psbp.tile([128, GW], f32, name="psb", tag="psb")
                ho = g * GW
                for j in range(2):
                    nc.tensor.matmul(
                        ps_a[:, j * NT:(j + 1) * NT], w2_sb,
                        h1a[:, ho + j * NT:ho + (j + 1) * NT],
                        start=True, stop=True)
                    nc.tensor.matmul(
                        ps_b[:, j * NT:(j + 1) * NT], w2_sb,
                        h1b[:, ho + j * NT:ho + (j + 1) * NT],
                        start=True, stop=True)
                nc.scalar.activation(h2a[:, ho:ho + GW], ps_a,
                                     AF.Relu, bias=b2_sb)
                if (2, blk, g) in _STEAL:
                    nc.scalar.activation(h2b[:, ho:ho + GW], ps_b,
                                         AF.Relu, bias=b2_sb)
                else:
                    nc.vector.tensor_scalar(h2b[:, ho:ho + GW], ps_b,
                                            b2_sb, 0.0,
                                            op0=ALU.add, op1=ALU.max)

            def l3_half(h2a, h2b, o_a, o_b, q):
                g, k = divmod(q, 2)
                if k == 0:
                    ps = psap.tile([128, GW], f32, name="psa", tag="psa")
                else:
                    ps = psbp.tile([128, GW], f32, name="psb", tag="psb")
                co = q * GW
                for j in range(2):
                    nc.tensor.matmul(
                        ps[0:OUTP, j * NT:(j + 1) * NT], w3_sb,
                        h2a[:, co + j * NT:co + (j + 1) * NT],
                        start=True, stop=True, tile_position=(0, 0))
                    nc.tensor.matmul(
                        ps[OUTP:128, j * NT:(j + 1) * NT], w3_sb,
                        h2b[:, co + j * NT:co + (j + 1) * NT],
                        start=True, stop=True, tile_position=(0, OUTP))
                oo = g * GW
                if k == 0:
                    nc.scalar.add(o_a[:, oo:oo + GW], ps, b3_sb)
                else:
                    nc.vector.tensor_scalar_add(o_b[:, oo:oo + GW], ps, b3_sb)
                return oo

            def out_dmas(o_a, o_b, b):
                base = b * BW
                dst = out_d[:, base:base + BW].rearrange(
                    "p (g t) -> p g t", t=2 * GW)
                for rows, orows in ((slice(0, OUT), slice(0, OUT)),
                                    (slice(OUT, 96),
                                     slice(OUTP, OUTP + OUT))):
                    nc.sync.dma_start(
                        out=dst[rows, :, 0:GW],
                        in_=o_a[orows, :].rearrange("p (g t) -> p g t", t=GW))
                    nc.sync.dma_start(
                        out=dst[rows, :, GW:2 * GW],
                        in_=o_b[orows, :].rearrange("p (g t) -> p g t", t=GW))

            loop = (tc.For_i(0, reps, 1, hint_engines=(mybir.EngineType.PE,))
                    if reps is not None else contextlib.nullcontext())
            with loop:
                # Compressed pipeline: one global tick stream; L2 trails L1
                # by 2 ticks, L3 halves trail their L2 producers by 2.
                # Global tick u = c * 8 + t; L1 tick u covers (block u//8,
                # group u%8); L2 at u covers L1's tick u-2; L3 half at u
                # covers h2 chunk of tick u-4 (a-half even, b-half odd).
                hs = {}   # block -> (h1a, h1b, h2a, h2b)
                os_ = {}  # block -> (o_a, o_b)
                NU = NB * 8
                for u in range(NU + 6):
                    c, t = divmod(u, 8)
                    if t == 0 and c < NB:
                        x_ts = []
                        for k in range(BW // XW):
                            x_t = xp.tile([128, XW], bf16, name="x_t")
                            c0 = c * BW + k * XW
                            nc.sync.dma_start(
                                out=x_t[0:64, :],
                                in_=x_d[0:64, c0:c0 + XW])
                            nc.sync.dma_start(
                                out=x_t[64:128, :],
                                in_=x_d[64:128, c0:c0 + XW])
                            x_ts.append(x_t)
                        hs[c] = (h1ap.tile([128, HW_], bf16, name="h1a"),
                                 h1bp.tile([128, HW_], bf16, name="h1b"),
                                 h2ap.tile([128, HW_], bf16, name="h2a"),
                                 h2bp.tile([128, HW_], bf16, name="h2b"),
                                 x_ts)
                        os_[c] = (oap.tile([128, 4096], bf16, name="o_a"),
                                  obp.tile([128, 4096], bf16, name="o_b"))
                    if u < NU:
                        h1a, h1b, _, _, x_ts = hs[c]
                        l1_group(x_ts, h1a, h1b, t, c)
                    u2 = u - 2
                    if 0 <= u2 < NU:
                        c2, t2 = divmod(u2, 8)
                        h1a, h1b, h2a, h2b, _ = hs[c2]
                        l2_group(h1a, h1b, h2a, h2b, t2, c2)
                    u3 = u - 4
                    if 0 <= u3 < NU:
                        c3, q = divmod(u3, 8)
                        _, _, h2a, h2b, _ = hs[c3]
                        o_a, o_b = os_[c3]
                        l3_half(h2a, h2b, o_a, o_b, q)
                        if c3 == NB - 1:
                            # last block: ship each chunk as it drains so the
                            # tail is just one small DMA deep
                            g3, k3 = divmod(q, 2)
                            o_t = o_a if k3 == 0 else o_b
                            oc = c3 * BW + g3 * 2 * GW + k3 * GW
                            nc.sync.dma_start(
                                out=out_d[0:OUT, oc:oc + GW],
                                in_=o_t[0:OUT, g3 * GW:(g3 + 1) * GW])
                            nc.sync.dma_start(
                                out=out_d[OUT:96, oc:oc + GW],
                                in_=o_t[OUTP:OUTP + OUT, g3 * GW:(g3 + 1) * GW])
                            if q == 7:
                                os_.pop(c3)
                                hs.pop(c3)
                        elif q == 7:
                            out_dmas(o_a, o_b, c3)
                            os_.pop(c3)
                            hs.pop(c3)

    nc.compile()
    return nc


def _get_nc():
    global _CACHED
    if _CACHED is None:
        _CACHED = _build_nc()
    return _CACHED


def _prep_member(x_e, W1_e, b1_e, W2_e, b2_e, W3_e, b3_e):
    xt = np.ascontiguousarray(np.asarray(x_e).T)      # [64, B] f32
    np.clip(xt[IN - AC:IN], -1.0, 1.0, out=xt[IN - AC:IN])
    X = np.empty((128, HB), dtype=BF16)
    X[0:64] = xt[:, :HB]
    X[64:128] = xt[:, HB:]

    w1p = np.empty((128, H), dtype=BF16)
    w1p[0:64] = W1_e
    w1p[64:128] = W1_e
    w2 = W2_e.astype(BF16)
    w3p = np.zeros((H, OUTP), dtype=BF16)
    w3p[:, :OUT] = W3_e
    b1v = np.ascontiguousarray(b1_e.astype(np.float32).reshape(H, 1))
    b2v = np.ascontiguousarray(b2_e.astype(np.float32).reshape(H, 1))
    b3v = np.zeros((128, 1), dtype=np.float32)
    b3v[0:OUT, 0] = b3_e
    b3v[OUTP:OUTP + OUT, 0] = b3_e
    return {"x": X, "w1p": w1p, "w2": w2, "w3p": w3p,
            "b1v": b1v, "b2v": b2v, "b3v": b3v}


def kernel(**inputs):
    from concourse.bass_utils import run_bass_kernel_spmd

    x = np.asarray(inputs["inputs"], dtype=np.float32).reshape(E, B, IN)
    W1 = np.asarray(inputs["W1"], dtype=np.float32)
    b1 = np.asarray(inputs["b1"], dtype=np.float32)
    W2 = np.asarray(inputs["W2"], dtype=np.float32)
    b2 = np.asarray(inputs["b2"], dtype=np.float32)
    W3 = np.asarray(inputs["W3"], dtype=np.float32)
    b3 = np.asarray(inputs["b3"], dtype=np.float32)

    in_maps = [
        _prep_member(x[e], W1[e], b1[e], W2[e], b2[e], W3[e], b3[e])
        for e in range(E)
    ]

    nc = _get_nc()
    res = run_bass_kernel_spmd(nc, in_maps, core_ids=list(range(E)))

    out = np.empty((E, B, OUT), dtype=np.float32)
    for e in range(E):
        dev = res.results[e]["out"]          # [96, HB] bf16
        out[e, :HB] = dev[0:OUT, :].T
        out[e, HB:] = dev[OUT:96, :].T
    return out
